# revision 1
# baseline (speedup 1.0000x reference)
"""Trainium2 Bass kernel for nn_Attend (l2-distance attention with zero-kv).

Reference computation (per b,h):
    k' = [0; k], v' = [0; v]                       (prepend zero kv)
    scores[i,j] = (2 q_i.k'_j - |q_i|^2 - |k'_j|^2) * (D+2)^-0.5
    causal: j <= i+1 in padded index space
    out = softmax(scores) @ v'

Kernel algebra: softmax is invariant to the per-row constant -scale*|q_i|^2,
so with p~[i,j] = exp(2*scale*q_i.k_j) and ek_j = exp(-scale*|k_j|^2) folded
into the PV stationary operand [V*ek | ek] (zero column contributes exp(0)=1
to the denominator only):
    out_i = (sum_j p~ (v_j ek_j)) / (1 + sum_j p~ ek_j)

Layout: scores are computed TRANSPOSED ([kv, q]) so P^T is directly the
moving operand of the PV matmul.  Heads are processed in PAIRS with K=128
(kT2 stacks both heads' k^T; q^T staged BLOCK-DIAGONALLY) to dodge the
half-rate moving-operand streaming at contraction <= 64.

exp is split across two engines to break the ACT bottleneck:
  - ACT: activation Exp (diagonal blocks + ~half the off-diagonal blocks)
  - DVE: Schraudolph bf16 exp: i16 = trunc(s*C1M + C2P) bit-cast to bf16
    approximates exp(2*scale*s) to ~1.8% rms; one tensor_scalar per block.
Causal masking touches only the 128-col mixed band of each diagonal block
(GPSIMD multiply); QK/exp/PV are column-restricted past the band, with the
diagonal blocks processed in DESCENDING r order so the PV accumulation
start/stop flags stay full-width.

Finalize avoids PE transposes: output stays transposed [d, q] on device
(host un-transposes); denominator+1 is broadcast across partitions by a
K=2 fp32r matmul against [den; ones], then DVE reciprocal + multiply.

Host-side prep (make_in_maps): bf16 cast + transposed/block-diagonal input
layouts + the [V|1] PV operand + mask constants.

Sharding: 32 (b,h) pairs -> 4 heads per core, 8 cores, pure data parallel.
"""

import sys

for _p in ("/opt/trn_rl_repo", "/root/.axon_site"):
    if _p not in sys.path:
        sys.path.insert(0, _p)

import numpy as np

B, H, N, D = 2, 16, 2048, 64
NCORES = 8
HPC = (B * H) // NCORES          # heads per core = 4
NPAIRS = HPC // 2
SCALE = float((D + 2) ** -0.5)   # augmented head dim, matches reference
NB = N // 128                    # kv blocks of 128 = 16
NQT = N // 512                   # q tiles of 512 = 4
LOG2E = 1.4426950408889634
C1M = float(2.0 * SCALE * 128.0 * LOG2E)
CSH = 0.0580                     # schraudolph correction (tuned, floor conv)
C2P = float(16256.0 - 128.0 * CSH + 0.5)  # +0.5: int16 convert truncates

_BUILT = {}


def _build(qk_dt="bfloat16", pv_dt="bfloat16", hpc=HPC, n=N):
    """Build + finalize the SPMD Bass program (one core's view)."""
    NB = n // 128
    NQT = n // 512
    import concourse.mybir as mybir
    import concourse.tile as tile
    from concourse import bacc

    f32 = mybir.dt.float32
    f32r = mybir.dt.float32r
    bf16 = mybir.dt.bfloat16
    i16 = mybir.dt.int16
    Exp = mybir.ActivationFunctionType.Exp
    Ln = mybir.ActivationFunctionType.Ln
    Identity = mybir.ActivationFunctionType.Identity
    add = mybir.AluOpType.add
    mult = mybir.AluOpType.mult

    nc = bacc.Bacc("TRN2", target_bir_lowering=False, debug=False, num_swdge_queues=4)
    qtp_p = nc.declare_dram_parameter("qtp", [NPAIRS, 128, n], bf16, isOutput=False)
    kt2_p = nc.declare_dram_parameter("kt2", [NPAIRS, 128, n], bf16, isOutput=False)
    vo_p = nc.declare_dram_parameter("vo", [hpc, 128, NB, 65], bf16, isOutput=False)
    kn_p = nc.declare_dram_parameter("kn", [hpc, 128, NB, 64], bf16, isOutput=False)
    mg_p = nc.declare_dram_parameter("mg", [128, 2, 128], bf16, isOutput=False)
    oneh_p = nc.declare_dram_parameter("oneh", [NQT, NQT, 64], f32r, isOutput=False)
    o_p = nc.declare_dram_parameter("out", [hpc, 64, n], f32, isOutput=True)

    # off-diagonal exp engine schedule: alternate DVE/ACT (tunable ratio)
    DVE_MOD = 2  # every DVE_MOD-th off-diag block goes to DVE... see below

    with tile.TileContext(nc) as tc:
        with (
            tc.tile_pool(name="const", bufs=1) as constp,
            tc.tile_pool(name="kqt", bufs=2) as kqtp,
            tc.tile_pool(name="prep", bufs=2) as prepp,
            tc.tile_pool(name="vop", bufs=2) as vop,
            tc.tile_pool(name="pt", bufs=6) as ptp,
            tc.tile_pool(name="fin", bufs=5) as finp,
            tc.tile_pool(name="nrmp", bufs=2) as nrmp,
            tc.tile_pool(name="densp", bufs=2) as densp,
            tc.tile_pool(name="ps_s", bufs=3, space="PSUM") as ps_s,
                        tc.tile_pool(name="ps_acc", bufs=1, space="PSUM") as ps_acc,
        ):
            mg = constp.tile([128, 2, 128], bf16, tag="mg")
            nc.sync.dma_start(out=mg[:], in_=mg_p[:])
            oneh = constp.tile([NQT, NQT, 64], f32r, tag="oneh")
            nc.sync.dma_start(out=oneh[:], in_=oneh_p[:])
            from concourse.masks import make_identity

            ident = constp.tile([128, 128], f32, tag="ident")
            make_identity(nc, ident[:])

            # ---- load + prep all pairs ------------------------------
            qTps, kT2s, vos = [], [], {}
            for pair in range(NPAIRS):
                hA, hB = 2 * pair, 2 * pair + 1
                qTp = kqtp.tile([128, n], bf16, tag="qTp", name=f"qTp_{pair}")
                kT2 = kqtp.tile([128, n], bf16, tag="kT2", name=f"kT2_{pair}")
                if pair == 0:
                    hn = n // 2
                    nc.sync.dma_start(out=qTp[:, 0:hn], in_=qtp_p[pair][:, 0:hn])
                    nc.sync.dma_start(out=qTp[:, hn:n], in_=qtp_p[pair][:, hn:n])
                    nc.scalar.dma_start(out=kT2[:, 0:hn], in_=kt2_p[pair][:, 0:hn])
                    nc.scalar.dma_start(
                        out=kT2[:, hn:n], in_=kt2_p[pair][:, hn:n]
                    )
                else:
                    nc.sync.dma_start(out=qTp[:], in_=qtp_p[pair])
                    nc.scalar.dma_start(out=kT2[:], in_=kt2_p[pair])
                qTps.append(qTp)
                kT2s.append(kT2)
                for h in (hA, hB):
                    kn = prepp.tile([128, NB, 64], bf16, tag="kn", name=f"kn_{h}")
                    vo = vop.tile([128, NB, 65], bf16, tag="vo", name=f"vo_{h}")
                    nc.gpsimd.dma_start(out=kn[:], in_=kn_p[h])
                    nc.gpsimd.dma_start(out=vo[:], in_=vo_p[h])
                    scr2 = prepp.tile([128, NB, 64], bf16, tag="scr2", name=f"s2_{h}")
                    nc.vector.tensor_mul(scr2[:], kn[:], kn[:])
                    ksqs = prepp.tile([128, NB], f32, tag="ksqs", name=f"ksq_{h}")
                    nc.vector.tensor_reduce(
                        ksqs[:], scr2[:], mybir.AxisListType.X, add
                    )
                    ek = prepp.tile([128, NB, 1], f32, tag="ek", name=f"ek_{h}")
                    nc.scalar.activation(ek[:, :, 0], ksqs[:], Exp, scale=-SCALE)
                    # vo *= ek (broadcast along the 65-wide last dim)
                    ekb = ek[:].broadcast_to([128, NB, 65])
                    nc.vector.scalar_tensor_tensor(
                        vo[:], vo[:], 1.0, ekb, mult, mult
                    )
                    vos[h] = vo

            # ---- finalize stage 2 (division via transposed recip) ----
            # one half-pair batch (2 q-tiles), split into 3 phases so the
            # PE pieces never wait on the DVE chain at an interleave point
            def stage2_phases(pair, half, densM2, accs2):
                hA, hB = 2 * pair, 2 * pair + 1
                tbase = 2 * half
                tag2 = f"{pair}_{half}"
                st = {}

                def phase_a():
                    denT = ps_s.tile([128, 8, 2], f32, tag="sp", name=f"dT{tag2}")
                    for c in range(8):
                        nc.tensor.matmul(
                            denT[:, c, :],
                            densM2[:, 128 * c : 128 * (c + 1)],
                            ident[0:2, 0:2],
                            is_transpose=True,
                            start=(c == 0),
                            stop=(c == 7),
                        )
                    rp = densp.tile([128, 8, 2], f32, tag="rp", name=f"rp{tag2}")
                    nc.vector.tensor_scalar_add(rp[:], denT[:], 1.0)
                    nc.vector.reciprocal(rp[:], rp[:])
                    st["rp"] = rp

                def phase_b():
                    rq = ps_s.tile([2, 1024], f32, tag="sp", name=f"rq{tag2}")
                    for c in range(8):
                        nc.tensor.matmul(
                            rq[:, 128 * c : 128 * (c + 1)],
                            st["rp"][:, c, :],
                            ident[:],
                            is_transpose=True,
                            start=(c in (0, 4)),
                            stop=(c in (3, 7)),
                        )
                    recs2 = densp.tile([2, 1024], f32r, tag="recs2", name=f"rc{tag2}")
                    nc.vector.tensor_copy(recs2[:], rq[:])
                    st["recs2"] = recs2

                def phase_c():
                    recs2 = st["recs2"]
                    for tl in range(2):
                        t = tbase + tl
                        db = ps_s.tile(
                            [64, 1024], f32, tag="sp", name=f"db{pair}_{t}"
                        )
                        for hh in range(2):
                            nc.tensor.matmul(
                                db[:, 512 * hh : 512 * (hh + 1)],
                                oneh[0:2, tl, :],
                                recs2[:, 512 * hh : 512 * (hh + 1)],
                                start=True,
                                stop=True,
                            )
                        nrm = nrmp.tile([64, 1024], f32, tag="nrm")
                        nc.vector.tensor_mul(nrm[:], accs2[tl][0:64, :], db[:])
                        nc.sync.dma_start(
                            out=o_p[hA][:, 512 * t : 512 * (t + 1)],
                            in_=nrm[:, 0:512],
                        )
                        nc.sync.dma_start(
                            out=o_p[hB][:, 512 * t : 512 * (t + 1)],
                            in_=nrm[:, 512:1024],
                        )

                return [(tag2, phase_a), (tag2, phase_b), (tag2, phase_c)]

            # ---- main flash loop ------------------------------------
            offdiag_ctr = 0
            phase_q = []  # pending stage2 phases, run ≤2 per interleave point
            for pair in range(NPAIRS):
                hA, hB = 2 * pair, 2 * pair + 1
                qTp, kT2 = qTps[pair], kT2s[pair]
                voA, voB = vos[hA], vos[hB]

                densMs = [
                    densp.tile([2, 1024], f32, tag="densM", name=f"dM{pair}_{h2}")
                    for h2 in range(2)
                ]
                accs_t = []
                for t in range(NQT):
                    if t == 2:
                        # this pair's first half (t0,t1 dens staged by now)
                        phase_q.extend(
                            stage2_phases(pair, 0, densMs[0], accs_t[0:2])
                        )
                    if phase_q:
                        gid, fn = phase_q.pop(0)
                        fn()
                        if phase_q and phase_q[0][0] != gid:
                            phase_q.pop(0)[1]()
                    nblk = 4 * (t + 1)
                    acc = ps_acc.tile([65, 1024], f32, tag="acc", name=f"ac{pair}_{t}")
                    # natural j order: start=True is full-width (j=0); the
                    # final stop is partial-width (r=3) which is fine —
                    # has_written state is consistent after j=0's full write.
                    # PV is deferred by 2 blocks so the PE FIFO has lookahead
                    # (QK j+1, j+2 run while exp(j) is in flight).
                    pvq = []

                    def emit_pv(jj):
                        rr = jj - 4 * t
                        cc0 = 128 * rr if rr >= 0 else 0
                        ptj = pvq_pt[jj]
                        nc.tensor.matmul(
                            acc[:, cc0:512],
                            voA[:, jj, :],
                            ptj[:, cc0:512],
                            start=(jj == 0),
                            stop=(jj == nblk - 1),
                        )
                        nc.tensor.matmul(
                            acc[:, 512 + cc0 : 1024],
                            voB[:, jj, :],
                            ptj[:, 512 + cc0 : 1024],
                            start=(jj == 0),
                            stop=(jj == nblk - 1),
                        )

                    pvq_pt = {}
                    for j in range(nblk):
                        r = j - 4 * t
                        diag = r >= 0
                        c0 = 128 * r if diag else 0  # column restriction
                        qsA = qTp[0:64, 512 * t + c0 : 512 * (t + 1)]
                        qsB = qTp[64:128, 512 * t + c0 : 512 * (t + 1)]
                        sp = ps_s.tile([128, 1024], f32, tag="sp")
                        nc.tensor.matmul(
                            sp[:, c0:512],
                            kT2[0:64, 128 * j : 128 * (j + 1)],
                            qsA,
                            start=True,
                            stop=True,
                        )
                        nc.tensor.matmul(
                            sp[:, 512 + c0 : 1024],
                            kT2[64:128, 128 * j : 128 * (j + 1)],
                            qsB,
                            start=True,
                            stop=True,
                        )
                        if len(pvq) >= 2:
                            emit_pv(pvq.pop(0))
                        pt = ptp.tile([128, 1024], bf16, tag="pt")
                        pvq_pt[j] = pt
                        sps = sp[:].rearrange("p (h c) -> p h c", h=2)[:, :, c0:512]
                        pts = pt[:].rearrange("p (h c) -> p h c", h=2)[:, :, c0:512]
                        if diag:
                            use_dve = False
                        else:
                            use_dve = (offdiag_ctr * 13) % 24 < 13
                            offdiag_ctr += 1
                        if use_dve:
                            nc.vector.tensor_scalar(
                                pts.bitcast(i16), sps, C1M, C2P, mult, add
                            )
                        else:
                            nc.scalar.activation(
                                pts, sps, Exp, scale=2.0 * SCALE
                            )
                        if diag:
                            # mask the 128-wide mixed band of both heads
                            band = pt[:].rearrange("p (h c) -> p h c", h=2)[
                                :, :, c0 : c0 + 128
                            ]
                            nc.gpsimd.tensor_tensor(band, band, mg[:], mult)
                        pvq.append(j)
                    for jj in pvq:
                        emit_pv(jj)

                    # ---- stash numerators + den row; free acc fast ------
                    # split halves across ACT+DVE so acc frees in ~0.7us
                    accs = finp.tile([65, 1024], f32, tag="accs", name=f"as{pair}_{t}")
                    nc.scalar.copy(accs[:, 0:512], acc[:, 0:512])
                    nc.vector.tensor_copy(accs[:, 512:1024], acc[:, 512:1024])
                    nc.sync.dma_start(
                        out=densMs[t // 2][t % 2 : t % 2 + 1, :], in_=accs[64:65, :]
                    )
                    accs_t.append(accs)

                phase_q.extend(stage2_phases(pair, 1, densMs[1], accs_t[2:4]))

            while phase_q:
                phase_q.pop(0)[1]()

    nc.finalize()
    return nc


def get_program(qk_dt="bfloat16", pv_dt="bfloat16"):
    key = (qk_dt, pv_dt)
    if key not in _BUILT:
        _BUILT[key] = _build(qk_dt, pv_dt)
    return _BUILT[key]


def make_in_maps(q, k, v, pv_dt="bfloat16"):
    """Host-side input staging: bf16 cast + transposed/blocked layouts."""
    import ml_dtypes

    bf = ml_dtypes.bfloat16
    qf = np.asarray(q, dtype=np.float32).reshape(B * H, N, D)
    kf = np.asarray(k, dtype=np.float32).reshape(B * H, N, D)
    vf = np.asarray(v, dtype=np.float32).reshape(B * H, N, D)

    j = np.arange(128)[:, None]
    cc = np.arange(128)[None, :]
    mg1 = (cc >= j).astype(bf)  # [128, 128]
    mg = np.ascontiguousarray(np.broadcast_to(mg1[:, None, :], (128, 2, 128)))
    oneh = np.ascontiguousarray(
        np.broadcast_to(np.eye(NQT, dtype=np.float32)[:, :, None], (NQT, NQT, 64))
    )

    maps = []
    for c in range(NCORES):
        base = c * HPC
        qtp = np.zeros((NPAIRS, 128, N), dtype=bf)
        kt2 = np.empty((NPAIRS, 128, N), dtype=bf)
        vo = np.empty((HPC, 128, NB, 65), dtype=bf)
        kn = np.empty((HPC, 128, NB, 64), dtype=bf)
        for p in range(NPAIRS):
            hA, hB = base + 2 * p, base + 2 * p + 1
            qtp[p, 0:64, :] = qf[hA].T.astype(bf)
            qtp[p, 64:128, :] = qf[hB].T.astype(bf)
            kt2[p, 0:64, :] = kf[hA].T.astype(bf)
            kt2[p, 64:128, :] = kf[hB].T.astype(bf)
        for hh in range(HPC):
            h = base + hh
            kh = kf[h].reshape(NB, 128, D).transpose(1, 0, 2)  # [128, NB, 64]
            vh = vf[h].reshape(NB, 128, D).transpose(1, 0, 2)
            kn[hh] = kh.astype(bf)
            vo[hh, :, :, 0:64] = vh.astype(bf)
            vo[hh, :, :, 64] = 1.0
        maps.append(
            {
                "qtp": qtp,
                "kt2": np.ascontiguousarray(kt2),
                "vo": vo,
                "kn": kn,
                "mg": mg,
                "oneh": oneh,
            }
        )
    return maps


def kernel(q, k, v):
    from concourse.bass_utils import run_bass_kernel_spmd

    nc = get_program()
    maps = make_in_maps(q, k, v)
    res = run_bass_kernel_spmd(nc, maps, list(range(NCORES)))
    out = np.concatenate(
        [res.results[c]["out"] for c in range(NCORES)], axis=0
    )  # [B*H, 64, N]
    return np.ascontiguousarray(out.transpose(0, 2, 1)).reshape(B, H, N, D)

